# revision 83
# baseline (speedup 1.0000x reference)
"""Trainium2 Bass kernel for nn_EncodingModule2d (vq_codebook), bf16 datapath.

Pipeline per batch item (pure data parallel, 1 item per NeuronCore, 8 cores):
  stem:   y = relu(BN2(conv_w @ x))       -- BN folded into weights on host
  vq:     dist2[n,k] = |y_n|^2 - 2<y_n, c_k> + |c_k|^2
          a = softmax_k(scales_k * dist2)
          agg[k,:] = sum_n a[n,k] (y_n - c_k)
  post:   z = mean_k relu(BN1(agg)); g = sigmoid(head_w @ z + head_b)
  out:    relu(x * (1 + g))

dtype strategy: bf16 for x, weights, y, ysq, a, head and the output (the PE
streams bf16 at 1 cyc/row at full clock vs ~2-4 for fp32/f32r, and the x/out
DMA halves); fp32 for the whole softmax path. The codebook scales s_k and
|c_k|^2 must NOT be rounded to bf16 (systematic logit error), so the distance
matmuls compute raw
  psum[k, n] = -2<c_k, y_n>   (k rows, stationary [-2c | 0] bf16)
  psum[32,n] = |y_n|^2        (q row, stationary [0 | ones] exact, ysqs moving)
and the (k,n)->(n,k) turn is a regular fp32 matmul lkn^T @ M with
M = [diag(s); s_row], which lands s_k*(-2<c,y> + |c|^2 + q) = logits in one
shot (|c|^2 enters as the per-partition bias of the psum->SBUF copy).
NB: is_transpose matmuls ignore the stationary values on HW (pure transpose),
hence the regular matmul here.

The y (d,n)->(n,d) relayout for the aggregation runs on the DMA transpose
crossbar (idle during compute), not the PE: out[p, j, d] = in[d, j*128+p]
into contiguous per-c-block planes; a third all-ones plane makes each agg
matmul also produce the a-rowsums needed for the centers correction.

Schedule: software-pipelined per 512-position slice with stem_rest lagging
stem_front by one slice and aggregation by two, so no engine queue head-of-
line blocks another; softmax goes per-chunk on the last slice to shorten the
tail. Dense bf16 warm-up matmuls on a zeroed tile ramp the HAM clock gate to
8/8 before the first real matmul; x streams in on three DMA queues.
"""

import os
import sys

for _p in ("/opt/trn_rl_repo",):
    if _p not in sys.path and os.path.isdir(_p):
        sys.path.insert(0, _p)

from contextlib import ExitStack

import numpy as np
import ml_dtypes

import concourse.bass as bass
import concourse.tile as tile
from concourse import bacc, mybir
from concourse.bass_utils import run_bass_kernel_spmd
from concourse.masks import make_identity

F32 = mybir.dt.float32
BF16 = mybir.dt.bfloat16
AF = mybir.ActivationFunctionType
ALU = mybir.AluOpType

B, D, H, W, K = 8, 256, 64, 64, 32
HW = H * W          # 4096 spatial positions
NB = D // 128       # 2 channel blocks of 128
NS = HW // 512      # 8 n-slices of 512
NCH = HW // 128     # 32 n-chunks of 128
CW = D + 1          # y_nd chunk width: 256 y + ones col
PK = K + 1          # dist psum partitions: 32 k rows + q row
PBW = D + 2 * PK + D    # packb cols: wT | ctA | onesA | head_w.T/K
EPS = 1e-5
N_CORES = 8


def _strided_cols(t, start, step, count, width):
    """AP over columns [start + i*step : start + i*step + width) of a 2D tile."""
    a = t[:, start : start + 1]
    return bass.AP(tensor=a.tensor, offset=a.offset, ap=[a.ap[0], [step, count], [1, width]])


def _build_program(has_bias2=False, dump=False):
    nc = bacc.Bacc("TRN2", target_bir_lowering=False, debug=False, num_devices=N_CORES)

    x_d = nc.dram_tensor("x", [D, HW], BF16, kind="ExternalInput").ap()
    # packb: [wT (256) | ctA (33) | onesA (33) | ident128] per c-block, bf16
    pb_d = nc.dram_tensor("packb", [D, PBW], BF16, kind="ExternalInput").ap()
    # packf: chv(bias2, s1, bb1, -head_b) fp32
    pf_d = nc.dram_tensor("packf", [D, 4], F32, kind="ExternalInput").ap()
    # smallf rows 0..32: [centers | c2 | pad | M33] fp32
    sf_d = nc.dram_tensor("smallf", [PK, D + 4 + K], F32, kind="ExternalInput").ap()
    out_d = nc.dram_tensor("out", [D, HW], BF16, kind="ExternalOutput").ap()
    if dump:
        dbg_y = nc.dram_tensor("dbg_y", [D, 512], BF16, kind="ExternalOutput").ap()
        dbg_lkn = nc.dram_tensor("dbg_lkn", [PK, HW], F32, kind="ExternalOutput").ap()
        dbg_a = nc.dram_tensor("dbg_a", [128, NCH * K], BF16, kind="ExternalOutput").ap()
        dbg_agg = nc.dram_tensor("dbg_agg", [32, D], F32, kind="ExternalOutput").ap()
        dbg_zg = nc.dram_tensor("dbg_zg", [128, 3 * NB], F32, kind="ExternalOutput").ap()
        dbg_lp = nc.dram_tensor("dbg_lp", [128, NCH * K], F32, kind="ExternalOutput").ap()
        dbg_es = nc.dram_tensor("dbg_es", [128, NCH * K], F32, kind="ExternalOutput").ap()
        dbg_ynd = nc.dram_tensor("dbg_ynd", [128, 3 * 512], BF16, kind="ExternalOutput").ap()

    with tile.TileContext(nc) as tc, ExitStack() as ctx:
        sb = ctx.enter_context(tc.tile_pool(name="sb", bufs=1))

        x_sb = sb.tile([128, NB, HW], BF16)
        packb = sb.tile([128, NB, PBW], BF16)
        packf = sb.tile([128, NB, 4], F32)
        smallf = sb.tile([PK, D + 4 + K], F32)

        # DMA order: x piece 0 + packb lead; x uses four queues so later
        # slices never starve; smallf early on scalar; packf (tail-only) last.
        qeng = [nc.sync, nc.scalar]
        # piece 512:1536 rides the gpsimd SWDGE queue (fast queues carry the
        # stem-critical piece 0 + weights, then stream the rest)
        pieces = [(0, 512), (512, 1536), (1536, 2560), (2560, 4096)]
        for q, (lo, hi) in enumerate(pieces):
            cs = slice(lo, hi)
            for c in range(NB):
                eng = nc.gpsimd if q == 1 else qeng[c]
                eng.dma_start(x_sb[:, c, cs], x_d[c * 128 : (c + 1) * 128, cs])
            if q == 0:
                nc.sync.dma_start(packb[:], pb_d.rearrange("(c p) m -> p c m", p=128))
                nc.scalar.dma_start(smallf[:], sf_d)
        nc.scalar.dma_start(packf[:], pf_d.rearrange("(c p) m -> p c m", p=128))

        wT = packb[:, :, 0:D]                     # (c, d->o) stem weights
        ctA = packb[:, :, D : D + PK]             # [-2*centers | zero col]
        onesA = packb[:, :, D + PK : D + 2 * PK]  # [zeros | ones col]
        hwT = packb[:, :, D + 2 * PK : D + 2 * PK + D]  # head_w.T / K (bf16)
        chv = packf[:, :, 0:4]                    # [bias2, s1, bb1, -head_b]
        ckd = smallf[0:K, 0:D]                    # centers (k,d) fp32
        c2col = smallf[:, D : D + 1]              # |c_k|^2 rows 0..31, 0 in row 32
        m33 = smallf[:, D + 4 : D + 4 + K]        # [diag(s); s_row] fp32

        ident = sb.tile([32, 32], F32)
        make_identity(nc, ident[:])

        # warm the exp table on ACT early (hidden under the x DMA)
        warm = sb.tile([128, 1], F32)
        nc.vector.memset(warm[:], 0.0)
        nc.scalar.activation(warm[:], warm[:], AF.Exp)

        # zero bf16 tile for dense PE warm-up matmuls (no DMA dependency)
        wb = sb.tile([128, 512], BF16)
        nc.vector.memset(wb[:], 0.0)

        # ---- big intermediates ----------------------------------------
        y_dn = sb.tile([128, NB, HW], BF16)       # relu(W'x): d on partitions
        ysq = sb.tile([128, NB, HW], BF16)        # y_dn^2
        ysqs = sb.tile([128, HW], BF16)           # ysq c0 + c1 (one q matmul)
        # y in (n, d) layout as c-block planes (contiguous DMA-transpose dests)
        # plus an all-ones plane whose matmul column yields the a rowsums
        y_nd = sb.tile([128, 3, HW], BF16)
        lkn = sb.tile([PK, HW], F32)              # [-2<c,y>+c2 ; q] in (k, n)
        esub = sb.tile([128, NCH * K], F32)       # logits - max
        e_sb = sb.tile([128, NCH * K], F32)       # exp(...)
        a_sb = sb.tile([128, NCH * K], BF16)      # softmax weights
        out_sb = sb.tile([128, NB, HW], BF16)

        maxt = sb.tile([128, NCH], F32)
        sumt = sb.tile([128, NCH], F32)
        rcp = sb.tile([128, NCH], F32)

        # ones plane of y_nd
        nc.vector.memset(y_nd[:, 2, :], 1.0)

        with ExitStack() as g_ctx:
            psG = g_ctx.enter_context(tc.tile_pool(name="psG", bufs=1, space="PSUM"))
            pagg = psG.tile([32, 3 * 128], F32)   # [agg (256) | rowsum_a cols]

            with ExitStack() as stem_ctx:
                psB = stem_ctx.enter_context(tc.tile_pool(name="psB", bufs=2, space="PSUM"))
                psK = stem_ctx.enter_context(tc.tile_pool(name="psK", bufs=2, space="PSUM"))
                psL = stem_ctx.enter_context(tc.tile_pool(name="psL", bufs=1, space="PSUM"))

                # unique psum slot per chunk (16 chunks of 32 per tile)
                logits_ps = [psL.tile([128, 512], F32, name=f"logits{i}") for i in range(2)]

                # HAM warm-up: dummy transposes of the on-chip identity keep
                # the PE clock ramping while x/packb stream in. They land in
                # psB tiles that the real stem matmuls overwrite (start=True).
                # HAM warm-up: dense 512-row bf16 matmuls on a zero tile keep
                # the PE clock ramping while x/packb stream in
                for i in range(8):
                    pWm = psB.tile([128, 512], F32, name="warm", tag="pB")
                    nc.tensor.matmul(pWm[:], wb[:, 0:128], wb[:],
                                     start=True, stop=True)

                def emit_softmax(g0, gn):
                    gs = slice(g0, g0 + gn)
                    cs = slice(g0 * K, (g0 + gn) * K)
                    lcs = slice((g0 % 16) * K, ((g0 % 16) + gn) * K)
                    lp3 = logits_ps[g0 // 16][:, lcs].rearrange("p (g k) -> p g k", g=gn)
                    nc.vector.tensor_reduce(out=maxt[:, gs], in_=lp3,
                                            axis=mybir.AxisListType.X, op=ALU.max)
                    mb = maxt[:, gs].rearrange("p (g u) -> p g u", u=1).broadcast_to((128, gn, K))
                    nc.vector.tensor_tensor(
                        out=esub[:, cs].rearrange("p (g k) -> p g k", g=gn),
                        in0=lp3, in1=mb, op=ALU.subtract)
                    nc.scalar.activation(e_sb[:, cs], esub[:, cs], AF.Exp)
                    nc.vector.tensor_reduce(out=sumt[:, gs],
                                            in_=e_sb[:, cs].rearrange("p (g k) -> p g k", g=gn),
                                            axis=mybir.AxisListType.X, op=ALU.add)
                    nc.vector.reciprocal(rcp[:, gs], sumt[:, gs])
                    rb = rcp[:, gs].rearrange("p (g u) -> p g u", u=1).broadcast_to((128, gn, K))
                    nc.vector.tensor_tensor(out=a_sb[:, cs].rearrange("p (g k) -> p g k", g=gn),
                                            in0=e_sb[:, cs].rearrange("p (g k) -> p g k", g=gn),
                                            in1=rb, op=ALU.mult)

                def stem_front(s):
                    """stem B matmuls + relu copies + squares for slice s."""
                    ns = slice(s * 512, (s + 1) * 512)
                    for o in range(NB):
                        pB = psB.tile([128, 512], F32)
                        for c in range(NB):
                            nc.tensor.matmul(
                                pB[:], wT[:, c, o * 128 : (o + 1) * 128],
                                x_sb[:, c, ns], start=(c == 0), stop=(c == NB - 1))
                        dst = y_dn[:, o, ns]
                        if o == 0:
                            if has_bias2:
                                nc.scalar.activation(dst, pB[:], AF.Relu,
                                                     bias=chv[:, o, 0:1])
                            else:
                                nc.scalar.activation(dst, pB[:], AF.Relu)
                        else:
                            if has_bias2:
                                nc.vector.tensor_scalar(
                                    out=dst, in0=pB[:], scalar1=chv[:, o, 0:1],
                                    scalar2=0.0, op0=ALU.add, op1=ALU.max)
                            else:
                                nc.vector.tensor_scalar_max(out=dst, in0=pB[:],
                                                            scalar1=0.0)
                def squares(s):
                    """ysq + ysqs for slice s (emitted after softmax(s-1) so
                    the DVE queue serves the critical softmax chain first)."""
                    ns = slice(s * 512, (s + 1) * 512)
                    nc.vector.tensor_tensor(out=ysq[:, 0, ns], in0=y_dn[:, 0, ns],
                                            in1=y_dn[:, 0, ns], op=ALU.mult)
                    nc.gpsimd.tensor_tensor(out=ysq[:, 1, ns], in0=y_dn[:, 1, ns],
                                            in1=y_dn[:, 1, ns], op=ALU.mult)
                    nc.vector.tensor_tensor(out=ysqs[:, ns], in0=ysq[:, 0, ns],
                                            in1=ysq[:, 1, ns], op=ALU.add)

                def stem_rest(s):
                    """transposes, distance, logits, softmax for slice s."""
                    ns = slice(s * 512, (s + 1) * 512)
                    # y (d,n) -> (n,d) via the DMA transpose crossbar, one
                    # contiguous [128, 4, 128] destination per c-block plane:
                    # out[p, j, d] = in[d, j*128 + p]
                    for c in range(NB):
                        nc.sync.dma_start_transpose(
                            y_nd[:, c, ns].rearrange("p (j d) -> p j d", d=128),
                            y_dn[:, c, ns])

                    # distance psum: [-2<c,y> rows ; q row]
                    pKt = psK.tile([PK, 512], F32)
                    nc.tensor.matmul(pKt[:], ctA[:, 0, :], y_dn[:, 0, ns],
                                     start=True, stop=False)
                    nc.tensor.matmul(pKt[:], ctA[:, 1, :], y_dn[:, 1, ns],
                                     start=False, stop=False)
                    nc.tensor.matmul(pKt[:], onesA[:, 0, :], ysqs[:, ns],
                                     start=False, stop=True)
                    # psum -> SBUF with the |c_k|^2 bias (row 32 bias 0)
                    dstl = lkn[:, ns]
                    nc.scalar.activation(dstl, pKt[:], AF.Identity, bias=c2col[:])

                    # logits: (k,n)->(n,k) via regular matmul with
                    # M = [diag(s); s_row]: lands s_k*(-2<c,y> + c2 + q)
                    for j in range(4 * s, 4 * s + 4):
                        nc.tensor.matmul(
                            logits_ps[j // 16][:, (j % 16) * K : (j % 16) * K + K],
                            lkn[:, j * 128 : (j + 1) * 128], m33[:],
                            start=True, stop=True)
                    if s < NS - 1:
                        emit_softmax(4 * s, 4)
                    else:
                        # last slice: per-chunk so aggregation starts sooner
                        for g in range(4 * s, 4 * s + 4):
                            emit_softmax(g, 1)

                def agg_emit(s):
                    if s == NS - 1:
                        # dense filler holds the clock gate while softmax of
                        # the last slice drains
                        for i in range(3):
                            pWm = psB.tile([128, 512], F32, name="warm", tag="pB")
                            nc.tensor.matmul(pWm[:], wb[:, 0:128], wb[:],
                                             start=True, stop=True)
                    for j in range(4 * s, 4 * s + 4):
                        nc.tensor.matmul(
                            pagg[:], a_sb[:, j * K : (j + 1) * K],
                            y_nd[:, :, j * 128 : (j + 1) * 128],
                            start=(j == 0), stop=(j == NCH - 1),
                            skip_group_check=True)

                # software pipeline: stem_rest lags stem_front by 1 slice,
                # aggregation lags 2, so no engine queue head blocks another.
                for it in range(NS + 2):
                    if it < NS:
                        stem_front(it)
                    if it >= 2:
                        agg_emit(it - 2)   # PE padding before dist(it-1)
                    if 1 <= it <= NS:
                        stem_rest(it - 1)
                    if it < NS:
                        squares(it)
                for i in range(3):
                    pWm = psB.tile([128, 512], F32, name="warm", tag="pB")
                    nc.tensor.matmul(pWm[:], wb[:, 0:128], wb[:],
                                     start=True, stop=True)

                if dump:
                    lpc = sb.tile([128, NCH * K], F32)
                    nc.vector.tensor_copy(lpc[:, 0:512], logits_ps[0][:])
                    nc.vector.tensor_copy(lpc[:, 512:1024], logits_ps[1][:])
                    nc.scalar.dma_start(dbg_lp[:], lpc[:])

            # ---- tail: BN1 + head + gating --------------------------------
            with ExitStack() as tail_ctx:
                psT = tail_ctx.enter_context(tc.tile_pool(name="psT", bufs=2, space="PSUM"))
                psH = tail_ctx.enter_context(tc.tile_pool(name="psH", bufs=2, space="PSUM"))

                # agg[k,d] = pagg[k,d] - rowsum_a[k] * centers[k,d]
                rsc = sb.tile([32, D], F32)
                nc.vector.tensor_scalar_mul(out=rsc[:], in0=ckd[:], scalar1=pagg[:, D : D + 1])
                agg_sb = sb.tile([32, D], F32)
                nc.vector.tensor_tensor(out=agg_sb[:], in0=pagg[:, 0:D], in1=rsc[:], op=ALU.subtract)

                # BN1 + relu + mean over k  ->  z per d-block
                z_t = sb.tile([128, NB], F32)
                t_sb = sb.tile([128, NB, K], F32)
                for b in range(NB):
                    pT = psT.tile([128, 32], F32)
                    nc.tensor.transpose(pT[:], agg_sb[:, b * 128 : (b + 1) * 128], ident[:])
                    nc.scalar.activation(t_sb[:, b, :], pT[:], AF.Relu,
                                         bias=chv[:, b, 2:3], scale=chv[:, b, 1:2],
                                         accum_out=z_t[:, b : b + 1])

                # head: gate = 1 + sigmoid(head_w @ z + head_b), bf16 matvec
                gate = sb.tile([128, NB], F32)
                eg = sb.tile([128, NB], F32)
                z_bf = sb.tile([128, NB], BF16)
                nc.vector.tensor_copy(z_bf[:], z_t[:])
                for o in range(NB):
                    pH = psH.tile([128, 1], F32)
                    for c in range(NB):
                        nc.tensor.matmul(pH[:], hwT[:, c, o * 128 : (o + 1) * 128],
                                         z_bf[:, c : c + 1],
                                         start=(c == 0), stop=(c == NB - 1))
                    nc.scalar.activation(eg[:, o : o + 1], pH[:], AF.Exp,
                                         bias=chv[:, o, 3:4], scale=-1.0)
                nc.vector.tensor_scalar_add(out=eg[:], in0=eg[:], scalar1=1.0)
                nc.vector.reciprocal(gate[:], eg[:])
                nc.vector.tensor_scalar_add(out=gate[:], in0=gate[:], scalar1=1.0)

                if dump:
                    nc.scalar.dma_start(dbg_es[:], esub[:])
                    for pl in range(3):
                        nc.scalar.dma_start(dbg_ynd[:, pl * 512 : (pl + 1) * 512],
                                            y_nd[:, pl, 0:512])
                    for c in range(NB):
                        nc.sync.dma_start(dbg_y[c * 128 : (c + 1) * 128, :],
                                          y_dn[:, c, 0:512])
                    nc.sync.dma_start(dbg_lkn[:], lkn[:])
                    nc.scalar.dma_start(dbg_a[:], a_sb[:])
                    nc.sync.dma_start(dbg_agg[:], agg_sb[:])
                    zg = sb.tile([128, 3 * NB], F32)
                    nc.vector.tensor_copy(zg[:, 0:NB], z_t[:])
                    nc.vector.tensor_copy(zg[:, NB : 2 * NB], eg[:])
                    nc.vector.tensor_copy(zg[:, 2 * NB : 3 * NB], gate[:])
                    nc.scalar.dma_start(dbg_zg[:], zg[:])

                # gating: out = relu(x * gate[d]); first pieces small so the
                # output DMA starts as early as possible
                # compute in 5 pieces per block (fast first DMA), DMA in 2
                # (fewer per-DMA overheads)
                gp = [(0, 512), (512, 1024), (1024, 2048), (2048, 3072), (3072, 4096)]
                for hh, (lo, hi) in enumerate(gp):
                    cs = slice(lo, hi)
                    for o in range(NB):
                        if o == 0 and hh in (1, 3):
                            nc.scalar.activation(out_sb[:, o, cs], x_sb[:, o, cs],
                                                 AF.Relu, scale=gate[:, o : o + 1])
                        else:
                            nc.vector.tensor_scalar(out=out_sb[:, o, cs], in0=x_sb[:, o, cs],
                                                    scalar1=gate[:, o : o + 1], scalar2=0.0,
                                                    op0=ALU.mult, op1=ALU.max)
                        if hh == 1:
                            qeng[o].dma_start(out_d[o * 128 : (o + 1) * 128, 0:1024],
                                              out_sb[:, o, 0:1024])
                        elif hh == 2:
                            qeng[o].dma_start(out_d[o * 128 : (o + 1) * 128, 1024:2048],
                                              out_sb[:, o, 1024:2048])
                        elif hh == 4:
                            qeng[o].dma_start(out_d[o * 128 : (o + 1) * 128, 2048:4096],
                                              out_sb[:, o, 2048:4096])

    nc.compile()
    return nc


_PROGRAM_CACHE = {}


def _get_program(has_bias2):
    key = bool(has_bias2)
    if key not in _PROGRAM_CACHE:
        _PROGRAM_CACHE[key] = _build_program(key)
    return _PROGRAM_CACHE[key]


def _host_params(conv_w, bn2_g, bn2_b, bn2_m, bn2_v, centers, scales,
                 bn1_g, bn1_b, bn1_m, bn1_v, head_w, head_b):
    scale2 = bn2_g / np.sqrt(bn2_v + EPS)
    wT = (conv_w * scale2[:, None]).T.astype(np.float32)             # (c, o)
    bias2 = (bn2_b - bn2_m * scale2).astype(np.float32)
    ctA = np.zeros((D, PK), np.float32)
    ctA[:, 0:K] = -2.0 * centers.T
    onesA = np.zeros((D, PK), np.float32)
    onesA[:, K] = 1.0
    hwT = (head_w.T / np.float32(K)).astype(np.float32)              # (d, o)
    packb = np.concatenate([wT, ctA, onesA, hwT], axis=1)
    packb = np.ascontiguousarray(packb.astype(ml_dtypes.bfloat16))   # (d, PBW)

    s1 = bn1_g / np.sqrt(bn1_v + EPS)
    bb1 = bn1_b - bn1_m * s1
    chv = np.stack([bias2, s1.astype(np.float32), bb1.astype(np.float32),
                    (-head_b).astype(np.float32)], axis=1).astype(np.float32)
    packf = np.ascontiguousarray(chv)

    smallf = np.zeros((PK, D + 4 + K), np.float32)
    smallf[0:K, 0:D] = centers
    smallf[0:K, D] = (centers * centers).sum(axis=1)
    smallf[0:K, D + 4 : D + 4 + K] = np.diag(scales.astype(np.float32))
    smallf[K, D + 4 : D + 4 + K] = scales
    return packb, packf, np.ascontiguousarray(smallf), bias2


def _ensure_profile_hook():
    """Register the axon NTFF profile hook if the image lacks antenv.axon_hooks."""
    import types

    if "antenv.axon_hooks" in sys.modules:
        return
    try:
        import antenv

        mod = types.ModuleType("antenv.axon_hooks")
        _hook = [None]
        mod.set_axon_ntff_profile_hook = lambda h: _hook.__setitem__(0, h)
        mod.get_axon_ntff_profile_hook = lambda: _hook[0]
        sys.modules["antenv.axon_hooks"] = mod
        antenv.axon_hooks = mod
        from trn_agent_boot.trn_boot import _ntff_profile_via_ctypes

        mod.set_axon_ntff_profile_hook(
            _ntff_profile_via_ctypes("/opt/axon/libaxon_pjrt.so"))
        import concourse.bass_utils as _bu

        _bu.upload_artifacts = lambda d: d  # no artifact store in this container
    except Exception as e:  # profiling is best-effort
        print(f"profile hook setup failed: {e}", file=sys.stderr)


def kernel(x, conv_w, bn2_g, bn2_b, bn2_m, bn2_v, centers, scales,
           bn1_g, bn1_b, bn1_m, bn1_v, head_w, head_b):
    x = np.asarray(x, dtype=np.float32)
    packb, packf, smallf, bias2 = _host_params(
        np.asarray(conv_w, np.float32), np.asarray(bn2_g, np.float32),
        np.asarray(bn2_b, np.float32), np.asarray(bn2_m, np.float32),
        np.asarray(bn2_v, np.float32), np.asarray(centers, np.float32),
        np.asarray(scales, np.float32), np.asarray(bn1_g, np.float32),
        np.asarray(bn1_b, np.float32), np.asarray(bn1_m, np.float32),
        np.asarray(bn1_v, np.float32), np.asarray(head_w, np.float32),
        np.asarray(head_b, np.float32))
    nc = _get_program(bool(np.abs(bias2).max() > 0))

    xb = np.ascontiguousarray(x.reshape(B, D, HW).astype(ml_dtypes.bfloat16))
    shared = {"packb": packb, "packf": packf, "smallf": smallf}
    in_maps = [dict(shared, x=xb[b]) for b in range(N_CORES)]

    trace = bool(int(os.environ.get("KERNEL_TRACE", "0")))
    kwargs = {}
    if trace:
        _ensure_profile_hook()
        tdir = os.environ.get("KERNEL_TRACE_DIR")
        if tdir:
            os.makedirs(tdir, exist_ok=True)
            kwargs["tmpdir"] = tdir
    res = run_bass_kernel_spmd(nc, in_maps, list(range(N_CORES)), trace=trace, **kwargs)
    if trace:
        kernel.last_exec_time_ns = res.exec_time_ns
        kernel.last_results = res
    out = np.stack([res.results[b]["out"].astype(np.float32).reshape(D, H, W)
                    for b in range(N_CORES)])
    return out


# revision 85
# speedup vs baseline: 1.0097x; 1.0097x over previous
"""Trainium2 Bass kernel for nn_EncodingModule2d (vq_codebook), bf16 datapath.

Pipeline per batch item (pure data parallel, 1 item per NeuronCore, 8 cores):
  stem:   y = relu(BN2(conv_w @ x))       -- BN folded into weights on host
  vq:     dist2[n,k] = |y_n|^2 - 2<y_n, c_k> + |c_k|^2
          a = softmax_k(scales_k * dist2)
          agg[k,:] = sum_n a[n,k] (y_n - c_k)
  post:   z = mean_k relu(BN1(agg)); g = sigmoid(head_w @ z + head_b)
  out:    relu(x * (1 + g))

dtype strategy: bf16 for x, weights, y, ysq, a, head and the output (the PE
streams bf16 at 1 cyc/row at full clock vs ~2-4 for fp32/f32r, and the x/out
DMA halves); fp32 for the whole softmax path. The codebook scales s_k and
|c_k|^2 must NOT be rounded to bf16 (systematic logit error), so the distance
matmuls compute raw
  psum[k, n] = -2<c_k, y_n>   (k rows, stationary [-2c | 0] bf16)
  psum[32,n] = |y_n|^2        (q row, stationary [0 | ones] exact, ysqs moving)
and the (k,n)->(n,k) turn is a regular fp32 matmul lkn^T @ M with
M = [diag(s); s_row], which lands s_k*(-2<c,y> + |c|^2 + q) = logits in one
shot (|c|^2 enters as the per-partition bias of the psum->SBUF copy).
NB: is_transpose matmuls ignore the stationary values on HW (pure transpose),
hence the regular matmul here.

The y (d,n)->(n,d) relayout for the aggregation runs on the DMA transpose
crossbar (idle during compute), not the PE: out[p, j, d] = in[d, j*128+p]
into contiguous per-c-block planes; a third all-ones plane makes each agg
matmul also produce the a-rowsums needed for the centers correction.

Schedule: software-pipelined per 512-position slice with stem_rest lagging
stem_front by one slice and aggregation by two, so no engine queue head-of-
line blocks another; softmax goes per-chunk on the last slice to shorten the
tail. Dense bf16 warm-up matmuls on a zeroed tile ramp the HAM clock gate to
8/8 before the first real matmul; x streams in on three DMA queues.
"""

import os
import sys

for _p in ("/opt/trn_rl_repo",):
    if _p not in sys.path and os.path.isdir(_p):
        sys.path.insert(0, _p)

from contextlib import ExitStack

import numpy as np
import ml_dtypes

import concourse.bass as bass
import concourse.tile as tile
from concourse import bacc, mybir
from concourse.bass_utils import run_bass_kernel_spmd
from concourse.masks import make_identity

F32 = mybir.dt.float32
BF16 = mybir.dt.bfloat16
AF = mybir.ActivationFunctionType
ALU = mybir.AluOpType

B, D, H, W, K = 8, 256, 64, 64, 32
HW = H * W          # 4096 spatial positions
NB = D // 128       # 2 channel blocks of 128
NS = HW // 512      # 8 n-slices of 512
NCH = HW // 128     # 32 n-chunks of 128
CW = D + 1          # y_nd chunk width: 256 y + ones col
PK = K + 1          # dist psum partitions: 32 k rows + q row
PBW = D + 2 * PK + D    # packb cols: wT | ctA | onesA | head_w.T/K
EPS = 1e-5
N_CORES = 8


def _strided_cols(t, start, step, count, width):
    """AP over columns [start + i*step : start + i*step + width) of a 2D tile."""
    a = t[:, start : start + 1]
    return bass.AP(tensor=a.tensor, offset=a.offset, ap=[a.ap[0], [step, count], [1, width]])


def _build_program(has_bias2=False, dump=False):
    nc = bacc.Bacc("TRN2", target_bir_lowering=False, debug=False, num_devices=N_CORES)

    x_d = nc.dram_tensor("x", [D, HW], BF16, kind="ExternalInput").ap()
    # packb: [wT (256) | ctA (33) | onesA (33) | ident128] per c-block, bf16
    pb_d = nc.dram_tensor("packb", [D, PBW], BF16, kind="ExternalInput").ap()
    # packf: chv(bias2, s1, bb1, -head_b) fp32
    pf_d = nc.dram_tensor("packf", [D, 4], F32, kind="ExternalInput").ap()
    # smallf rows 0..32: [centers | c2 | pad | M33] fp32
    sf_d = nc.dram_tensor("smallf", [PK, D + 4 + K], F32, kind="ExternalInput").ap()
    out_d = nc.dram_tensor("out", [D, HW], BF16, kind="ExternalOutput").ap()
    if dump:
        dbg_y = nc.dram_tensor("dbg_y", [D, 512], BF16, kind="ExternalOutput").ap()
        dbg_lkn = nc.dram_tensor("dbg_lkn", [PK, HW], F32, kind="ExternalOutput").ap()
        dbg_a = nc.dram_tensor("dbg_a", [128, NCH * K], BF16, kind="ExternalOutput").ap()
        dbg_agg = nc.dram_tensor("dbg_agg", [32, D], F32, kind="ExternalOutput").ap()
        dbg_zg = nc.dram_tensor("dbg_zg", [128, 3 * NB], F32, kind="ExternalOutput").ap()
        dbg_lp = nc.dram_tensor("dbg_lp", [128, NCH * K], F32, kind="ExternalOutput").ap()
        dbg_es = nc.dram_tensor("dbg_es", [128, NCH * K], F32, kind="ExternalOutput").ap()
        dbg_ynd = nc.dram_tensor("dbg_ynd", [128, 3 * 512], BF16, kind="ExternalOutput").ap()

    with tile.TileContext(nc) as tc, ExitStack() as ctx:
        sb = ctx.enter_context(tc.tile_pool(name="sb", bufs=1))

        x_sb = sb.tile([128, NB, HW], BF16)
        packb = sb.tile([128, NB, PBW], BF16)
        packf = sb.tile([128, NB, 4], F32)
        smallf = sb.tile([PK, D + 4 + K], F32)

        # DMA order: x piece 0 + packb lead; x uses four queues so later
        # slices never starve; smallf early on scalar; packf (tail-only) last.
        qeng = [nc.sync, nc.scalar]
        # piece 512:1536 rides the gpsimd SWDGE queue (fast queues carry the
        # stem-critical piece 0 + weights, then stream the rest)
        pieces = [(0, 512), (512, 1536), (1536, 2560), (2560, 4096)]
        for q, (lo, hi) in enumerate(pieces):
            cs = slice(lo, hi)
            for c in range(NB):
                eng = nc.gpsimd if q == 1 else qeng[c]
                eng.dma_start(x_sb[:, c, cs], x_d[c * 128 : (c + 1) * 128, cs])
            if q == 0:
                nc.sync.dma_start(packb[:], pb_d.rearrange("(c p) m -> p c m", p=128))
                nc.scalar.dma_start(smallf[:], sf_d)
        nc.scalar.dma_start(packf[:], pf_d.rearrange("(c p) m -> p c m", p=128))

        wT = packb[:, :, 0:D]                     # (c, d->o) stem weights
        ctA = packb[:, :, D : D + PK]             # [-2*centers | zero col]
        onesA = packb[:, :, D + PK : D + 2 * PK]  # [zeros | ones col]
        hwT = packb[:, :, D + 2 * PK : D + 2 * PK + D]  # head_w.T / K (bf16)
        chv = packf[:, :, 0:4]                    # [bias2, s1, bb1, -head_b]
        ckd = smallf[0:K, 0:D]                    # centers (k,d) fp32
        c2col = smallf[:, D : D + 1]              # |c_k|^2 rows 0..31, 0 in row 32
        m33 = smallf[:, D + 4 : D + 4 + K]        # [diag(s); s_row] fp32

        ident = sb.tile([32, 32], F32)
        make_identity(nc, ident[:])

        # warm the exp table on ACT early (hidden under the x DMA)
        warm = sb.tile([128, 1], F32)
        nc.vector.memset(warm[:], 0.0)
        nc.scalar.activation(warm[:], warm[:], AF.Exp)

        # zero bf16 tile for dense PE warm-up matmuls (no DMA dependency)
        wb = sb.tile([128, 512], BF16)
        nc.vector.memset(wb[:], 0.0)

        # ---- big intermediates ----------------------------------------
        y_dn = sb.tile([128, NB, HW], BF16)       # relu(W'x): d on partitions
        ysq = sb.tile([128, NB, HW], BF16)        # y_dn^2
        ysqs = sb.tile([128, HW], BF16)           # ysq c0 + c1 (one q matmul)
        # y in (n, d) layout as c-block planes (contiguous DMA-transpose dests)
        # plus an all-ones plane whose matmul column yields the a rowsums
        y_nd = sb.tile([128, 3, HW], BF16)
        lkn = sb.tile([PK, HW], F32)              # [-2<c,y>+c2 ; q] in (k, n)
        esub = sb.tile([128, NCH * K], F32)       # logits - max
        e_sb = sb.tile([128, NCH * K], F32)       # exp(...)
        a_sb = sb.tile([128, NCH * K], BF16)      # softmax weights
        out_sb = sb.tile([128, NB, HW], BF16)

        maxt = sb.tile([128, NCH], F32)
        sumt = sb.tile([128, NCH], F32)
        rcp = sb.tile([128, NCH], F32)

        # ones plane of y_nd
        nc.vector.memset(y_nd[:, 2, :], 1.0)

        with ExitStack() as g_ctx:
            psG = g_ctx.enter_context(tc.tile_pool(name="psG", bufs=1, space="PSUM"))
            pagg = psG.tile([32, 3 * 128], F32)   # [agg (256) | rowsum_a cols]

            with ExitStack() as stem_ctx:
                psB = stem_ctx.enter_context(tc.tile_pool(name="psB", bufs=2, space="PSUM"))
                psK = stem_ctx.enter_context(tc.tile_pool(name="psK", bufs=2, space="PSUM"))
                psL = stem_ctx.enter_context(tc.tile_pool(name="psL", bufs=1, space="PSUM"))

                # unique psum slot per chunk (16 chunks of 32 per tile)
                logits_ps = [psL.tile([128, 512], F32, name=f"logits{i}") for i in range(2)]

                # HAM warm-up: dummy transposes of the on-chip identity keep
                # the PE clock ramping while x/packb stream in. They land in
                # psB tiles that the real stem matmuls overwrite (start=True).
                # HAM warm-up: dense 512-row bf16 matmuls on a zero tile keep
                # the PE clock ramping while x/packb stream in
                for i in range(8):
                    pWm = psB.tile([128, 512], F32, name="warm", tag="pB")
                    nc.tensor.matmul(pWm[:], wb[:, 0:128], wb[:],
                                     start=True, stop=True)

                def emit_softmax(g0, gn):
                    gs = slice(g0, g0 + gn)
                    cs = slice(g0 * K, (g0 + gn) * K)
                    lcs = slice((g0 % 16) * K, ((g0 % 16) + gn) * K)
                    lp3 = logits_ps[g0 // 16][:, lcs].rearrange("p (g k) -> p g k", g=gn)
                    nc.vector.tensor_reduce(out=maxt[:, gs], in_=lp3,
                                            axis=mybir.AxisListType.X, op=ALU.max)
                    mb = maxt[:, gs].rearrange("p (g u) -> p g u", u=1).broadcast_to((128, gn, K))
                    nc.vector.tensor_tensor(
                        out=esub[:, cs].rearrange("p (g k) -> p g k", g=gn),
                        in0=lp3, in1=mb, op=ALU.subtract)
                    nc.scalar.activation(e_sb[:, cs], esub[:, cs], AF.Exp)
                    nc.vector.tensor_reduce(out=sumt[:, gs],
                                            in_=e_sb[:, cs].rearrange("p (g k) -> p g k", g=gn),
                                            axis=mybir.AxisListType.X, op=ALU.add)
                    nc.vector.reciprocal(rcp[:, gs], sumt[:, gs])
                    rb = rcp[:, gs].rearrange("p (g u) -> p g u", u=1).broadcast_to((128, gn, K))
                    nc.vector.tensor_tensor(out=a_sb[:, cs].rearrange("p (g k) -> p g k", g=gn),
                                            in0=e_sb[:, cs].rearrange("p (g k) -> p g k", g=gn),
                                            in1=rb, op=ALU.mult)

                def stem_front(s):
                    """stem B matmuls + relu copies + squares for slice s."""
                    ns = slice(s * 512, (s + 1) * 512)
                    for o in range(NB):
                        pB = psB.tile([128, 512], F32)
                        for c in range(NB):
                            nc.tensor.matmul(
                                pB[:], wT[:, c, o * 128 : (o + 1) * 128],
                                x_sb[:, c, ns], start=(c == 0), stop=(c == NB - 1))
                        dst = y_dn[:, o, ns]
                        if o == 0:
                            if has_bias2:
                                nc.scalar.activation(dst, pB[:], AF.Relu,
                                                     bias=chv[:, o, 0:1])
                            else:
                                nc.scalar.activation(dst, pB[:], AF.Relu)
                        else:
                            if has_bias2:
                                nc.vector.tensor_scalar(
                                    out=dst, in0=pB[:], scalar1=chv[:, o, 0:1],
                                    scalar2=0.0, op0=ALU.add, op1=ALU.max)
                            else:
                                nc.vector.tensor_scalar_max(out=dst, in0=pB[:],
                                                            scalar1=0.0)
                def squares(s):
                    """ysq + ysqs for slice s (emitted after softmax(s-1) so
                    the DVE queue serves the critical softmax chain first)."""
                    ns = slice(s * 512, (s + 1) * 512)
                    nc.vector.tensor_tensor(out=ysq[:, 0, ns], in0=y_dn[:, 0, ns],
                                            in1=y_dn[:, 0, ns], op=ALU.mult)
                    nc.gpsimd.tensor_tensor(out=ysq[:, 1, ns], in0=y_dn[:, 1, ns],
                                            in1=y_dn[:, 1, ns], op=ALU.mult)
                    nc.vector.tensor_tensor(out=ysqs[:, ns], in0=ysq[:, 0, ns],
                                            in1=ysq[:, 1, ns], op=ALU.add)

                def stem_rest(s):
                    """transposes, distance, logits, softmax for slice s."""
                    ns = slice(s * 512, (s + 1) * 512)
                    # y (d,n) -> (n,d) via the DMA transpose crossbar, one
                    # contiguous [128, 4, 128] destination per c-block plane:
                    # out[p, j, d] = in[d, j*128 + p]
                    for c in range(NB):
                        nc.sync.dma_start_transpose(
                            y_nd[:, c, ns].rearrange("p (j d) -> p j d", d=128),
                            y_dn[:, c, ns])

                    # distance psum: [-2<c,y> rows ; q row]
                    pKt = psK.tile([PK, 512], F32)
                    nc.tensor.matmul(pKt[:], ctA[:, 0, :], y_dn[:, 0, ns],
                                     start=True, stop=False)
                    nc.tensor.matmul(pKt[:], ctA[:, 1, :], y_dn[:, 1, ns],
                                     start=False, stop=False)
                    nc.tensor.matmul(pKt[:], onesA[:, 0, :], ysqs[:, ns],
                                     start=False, stop=True)
                    # psum -> SBUF with the |c_k|^2 bias (row 32 bias 0)
                    dstl = lkn[:, ns]
                    nc.scalar.activation(dstl, pKt[:], AF.Identity, bias=c2col[:])

                    # logits: (k,n)->(n,k) via regular matmul with
                    # M = [diag(s); s_row]: lands s_k*(-2<c,y> + c2 + q)
                    for j in range(4 * s, 4 * s + 4):
                        nc.tensor.matmul(
                            logits_ps[j // 16][:, (j % 16) * K : (j % 16) * K + K],
                            lkn[:, j * 128 : (j + 1) * 128], m33[:],
                            start=True, stop=True)
                    if s < NS - 1:
                        emit_softmax(4 * s, 4)
                    else:
                        # last slice: per-chunk so aggregation starts sooner
                        for g in range(4 * s, 4 * s + 4):
                            emit_softmax(g, 1)

                def agg_emit(s):
                    for j in range(4 * s, 4 * s + 4):
                        nc.tensor.matmul(
                            pagg[:], a_sb[:, j * K : (j + 1) * K],
                            y_nd[:, :, j * 128 : (j + 1) * 128],
                            start=(j == 0), stop=(j == NCH - 1),
                            skip_group_check=True)

                # software pipeline: stem_rest lags stem_front by 1 slice,
                # aggregation lags 2, so no engine queue head blocks another.
                for it in range(NS + 2):
                    if it < NS:
                        stem_front(it)
                    if it >= 2:
                        agg_emit(it - 2)   # PE padding before dist(it-1)
                    if 1 <= it <= NS:
                        stem_rest(it - 1)
                    if it < NS:
                        squares(it)

                if dump:
                    lpc = sb.tile([128, NCH * K], F32)
                    nc.vector.tensor_copy(lpc[:, 0:512], logits_ps[0][:])
                    nc.vector.tensor_copy(lpc[:, 512:1024], logits_ps[1][:])
                    nc.scalar.dma_start(dbg_lp[:], lpc[:])

            # ---- tail: BN1 + head + gating --------------------------------
            with ExitStack() as tail_ctx:
                psT = tail_ctx.enter_context(tc.tile_pool(name="psT", bufs=2, space="PSUM"))
                psH = tail_ctx.enter_context(tc.tile_pool(name="psH", bufs=2, space="PSUM"))

                # agg[k,d] = pagg[k,d] - rowsum_a[k] * centers[k,d]
                rsc = sb.tile([32, D], F32)
                nc.vector.tensor_scalar_mul(out=rsc[:], in0=ckd[:], scalar1=pagg[:, D : D + 1])
                agg_sb = sb.tile([32, D], F32)
                nc.vector.tensor_tensor(out=agg_sb[:], in0=pagg[:, 0:D], in1=rsc[:], op=ALU.subtract)

                # BN1 + relu + mean over k  ->  z per d-block
                z_t = sb.tile([128, NB], F32)
                t_sb = sb.tile([128, NB, K], F32)
                for b in range(NB):
                    pT = psT.tile([128, 32], F32)
                    nc.tensor.transpose(pT[:], agg_sb[:, b * 128 : (b + 1) * 128], ident[:])
                    nc.scalar.activation(t_sb[:, b, :], pT[:], AF.Relu,
                                         bias=chv[:, b, 2:3], scale=chv[:, b, 1:2],
                                         accum_out=z_t[:, b : b + 1])

                # head: gate = 1 + sigmoid(head_w @ z + head_b), bf16 matvec
                gate = sb.tile([128, NB], F32)
                eg = sb.tile([128, NB], F32)
                z_bf = sb.tile([128, NB], BF16)
                nc.vector.tensor_copy(z_bf[:], z_t[:])
                for o in range(NB):
                    pH = psH.tile([128, 1], F32)
                    for c in range(NB):
                        nc.tensor.matmul(pH[:], hwT[:, c, o * 128 : (o + 1) * 128],
                                         z_bf[:, c : c + 1],
                                         start=(c == 0), stop=(c == NB - 1))
                    nc.scalar.activation(eg[:, o : o + 1], pH[:], AF.Exp,
                                         bias=chv[:, o, 3:4], scale=-1.0)
                nc.vector.tensor_scalar_add(out=eg[:], in0=eg[:], scalar1=1.0)
                nc.vector.reciprocal(gate[:], eg[:])
                nc.vector.tensor_scalar_add(out=gate[:], in0=gate[:], scalar1=1.0)

                if dump:
                    nc.scalar.dma_start(dbg_es[:], esub[:])
                    for pl in range(3):
                        nc.scalar.dma_start(dbg_ynd[:, pl * 512 : (pl + 1) * 512],
                                            y_nd[:, pl, 0:512])
                    for c in range(NB):
                        nc.sync.dma_start(dbg_y[c * 128 : (c + 1) * 128, :],
                                          y_dn[:, c, 0:512])
                    nc.sync.dma_start(dbg_lkn[:], lkn[:])
                    nc.scalar.dma_start(dbg_a[:], a_sb[:])
                    nc.sync.dma_start(dbg_agg[:], agg_sb[:])
                    zg = sb.tile([128, 3 * NB], F32)
                    nc.vector.tensor_copy(zg[:, 0:NB], z_t[:])
                    nc.vector.tensor_copy(zg[:, NB : 2 * NB], eg[:])
                    nc.vector.tensor_copy(zg[:, 2 * NB : 3 * NB], gate[:])
                    nc.scalar.dma_start(dbg_zg[:], zg[:])

                # gating: out = relu(x * gate[d]); first pieces small so the
                # output DMA starts as early as possible
                # compute in 5 pieces per block (fast first DMA), DMA in 2
                # (fewer per-DMA overheads)
                gp = [(0, 512), (512, 1024), (1024, 2048), (2048, 3072), (3072, 4096)]
                for hh, (lo, hi) in enumerate(gp):
                    cs = slice(lo, hi)
                    for o in range(NB):
                        if o == 0 and hh in (1, 3):
                            nc.scalar.activation(out_sb[:, o, cs], x_sb[:, o, cs],
                                                 AF.Relu, scale=gate[:, o : o + 1])
                        else:
                            nc.vector.tensor_scalar(out=out_sb[:, o, cs], in0=x_sb[:, o, cs],
                                                    scalar1=gate[:, o : o + 1], scalar2=0.0,
                                                    op0=ALU.mult, op1=ALU.max)
                        if hh == 1:
                            qeng[o].dma_start(out_d[o * 128 : (o + 1) * 128, 0:1024],
                                              out_sb[:, o, 0:1024])
                        elif hh == 2:
                            qeng[o].dma_start(out_d[o * 128 : (o + 1) * 128, 1024:2048],
                                              out_sb[:, o, 1024:2048])
                        elif hh == 4:
                            qeng[o].dma_start(out_d[o * 128 : (o + 1) * 128, 2048:4096],
                                              out_sb[:, o, 2048:4096])

    nc.compile()
    return nc


_PROGRAM_CACHE = {}


def _get_program(has_bias2):
    key = bool(has_bias2)
    if key not in _PROGRAM_CACHE:
        _PROGRAM_CACHE[key] = _build_program(key)
    return _PROGRAM_CACHE[key]


def _host_params(conv_w, bn2_g, bn2_b, bn2_m, bn2_v, centers, scales,
                 bn1_g, bn1_b, bn1_m, bn1_v, head_w, head_b):
    scale2 = bn2_g / np.sqrt(bn2_v + EPS)
    wT = (conv_w * scale2[:, None]).T.astype(np.float32)             # (c, o)
    bias2 = (bn2_b - bn2_m * scale2).astype(np.float32)
    ctA = np.zeros((D, PK), np.float32)
    ctA[:, 0:K] = -2.0 * centers.T
    onesA = np.zeros((D, PK), np.float32)
    onesA[:, K] = 1.0
    hwT = (head_w.T / np.float32(K)).astype(np.float32)              # (d, o)
    packb = np.concatenate([wT, ctA, onesA, hwT], axis=1)
    packb = np.ascontiguousarray(packb.astype(ml_dtypes.bfloat16))   # (d, PBW)

    s1 = bn1_g / np.sqrt(bn1_v + EPS)
    bb1 = bn1_b - bn1_m * s1
    chv = np.stack([bias2, s1.astype(np.float32), bb1.astype(np.float32),
                    (-head_b).astype(np.float32)], axis=1).astype(np.float32)
    packf = np.ascontiguousarray(chv)

    smallf = np.zeros((PK, D + 4 + K), np.float32)
    smallf[0:K, 0:D] = centers
    smallf[0:K, D] = (centers * centers).sum(axis=1)
    smallf[0:K, D + 4 : D + 4 + K] = np.diag(scales.astype(np.float32))
    smallf[K, D + 4 : D + 4 + K] = scales
    return packb, packf, np.ascontiguousarray(smallf), bias2


def _ensure_profile_hook():
    """Register the axon NTFF profile hook if the image lacks antenv.axon_hooks."""
    import types

    if "antenv.axon_hooks" in sys.modules:
        return
    try:
        import antenv

        mod = types.ModuleType("antenv.axon_hooks")
        _hook = [None]
        mod.set_axon_ntff_profile_hook = lambda h: _hook.__setitem__(0, h)
        mod.get_axon_ntff_profile_hook = lambda: _hook[0]
        sys.modules["antenv.axon_hooks"] = mod
        antenv.axon_hooks = mod
        from trn_agent_boot.trn_boot import _ntff_profile_via_ctypes

        mod.set_axon_ntff_profile_hook(
            _ntff_profile_via_ctypes("/opt/axon/libaxon_pjrt.so"))
        import concourse.bass_utils as _bu

        _bu.upload_artifacts = lambda d: d  # no artifact store in this container
    except Exception as e:  # profiling is best-effort
        print(f"profile hook setup failed: {e}", file=sys.stderr)


def kernel(x, conv_w, bn2_g, bn2_b, bn2_m, bn2_v, centers, scales,
           bn1_g, bn1_b, bn1_m, bn1_v, head_w, head_b):
    x = np.asarray(x, dtype=np.float32)
    packb, packf, smallf, bias2 = _host_params(
        np.asarray(conv_w, np.float32), np.asarray(bn2_g, np.float32),
        np.asarray(bn2_b, np.float32), np.asarray(bn2_m, np.float32),
        np.asarray(bn2_v, np.float32), np.asarray(centers, np.float32),
        np.asarray(scales, np.float32), np.asarray(bn1_g, np.float32),
        np.asarray(bn1_b, np.float32), np.asarray(bn1_m, np.float32),
        np.asarray(bn1_v, np.float32), np.asarray(head_w, np.float32),
        np.asarray(head_b, np.float32))
    nc = _get_program(bool(np.abs(bias2).max() > 0))

    xb = np.ascontiguousarray(x.reshape(B, D, HW).astype(ml_dtypes.bfloat16))
    shared = {"packb": packb, "packf": packf, "smallf": smallf}
    in_maps = [dict(shared, x=xb[b]) for b in range(N_CORES)]

    trace = bool(int(os.environ.get("KERNEL_TRACE", "0")))
    kwargs = {}
    if trace:
        _ensure_profile_hook()
        tdir = os.environ.get("KERNEL_TRACE_DIR")
        if tdir:
            os.makedirs(tdir, exist_ok=True)
            kwargs["tmpdir"] = tdir
    res = run_bass_kernel_spmd(nc, in_maps, list(range(N_CORES)), trace=trace, **kwargs)
    if trace:
        kernel.last_exec_time_ns = res.exec_time_ns
        kernel.last_results = res
    out = np.stack([res.results[b]["out"].astype(np.float32).reshape(D, H, W)
                    for b in range(N_CORES)])
    return out
